# revision 19
# baseline (speedup 1.0000x reference)
"""Trainium2 Bass kernel for nn_LowRankGNN (vq_codebook).

Math restructure (exact algebra, host-side weight folding):
  - Only edges with dst < B contribute to the output (agg[:B] is all that's used).
  - segment_sum(w_e * (x_input @ Wc)[src], dst)[:B] @ Wt
      == segment_sum(w_e * x_input[src], dst)[:B] @ (Wc @ Wt)
    so per layer:  out = seg @ Wct + h @ Ws + bias,  Wct = Wc@Wt,
    bias = bc@Wt + bt + bs,  seg = segment_sum over dst<B edges of w_e*x_input[src].

Sharding: data-parallel over the B mini-batch rows (dst blocks of B/8 per core).
Each core handles the edges targeting its dst rows.  Per layer, per core:
  - msgs gather: indirect-DMA rows of x_input for its edges
      src <  B  -> rows from a compact exchanged h-table (AllToAll between layers)
      src >= B  -> 4 per-branch codebook row-halves (vq gather), indices precomputed
  - scatter:  one-hot matmul on the PE: segT[f,d] += msgs[e,f].T @ SelT[e,d]
      (SelT holds w_e at [e, dst_col]; host-precomputed, resident in SBUF, reused 3x)
  - dense:    out[d,f] = segT.T @ Wct + hT.T @ Ws + ones (x) bias   (PE, row-major
      output; hT slices come from bf16 DMA-transpose loads of the local h table)
  - exchange: compact AllToAll of only the h rows other cores' edges reference.
Compute dtype bf16 (PE), accumulation fp32 (PSUM); final output fp32.
"""

import math

import ml_dtypes
import numpy as np

import concourse.bass as bass
import concourse.mybir as mybir
import concourse.tile as tile
from concourse import bacc
from concourse.bass_utils import run_bass_kernel_spmd

# ---------------------------------------------------------------- problem config
CFG = dict(
    L=3, NBR=4, D=64, M=2048, NN=500000,
    B=20000, NF=60000, E=640000, C=256,
    NCORES=8, BLK=128, WIN_BLOCKS=4,
)

BF16 = ml_dtypes.bfloat16


def _derived(cfg):
    d = dict(cfg)
    d["NODES"] = cfg["B"] + cfg["NF"]
    d["BC"] = cfg["B"] // cfg["NCORES"]            # per-core dst rows
    d["NBLK"] = math.ceil(d["BC"] / cfg["BLK"])    # dst blocks per core
    d["BCP"] = d["NBLK"] * cfg["BLK"]              # padded per-core rows
    return d


# ---------------------------------------------------------------- host preprocessing
def make_plan(cfg, first_order_idx, edge_src, edge_dst, edge_weight, c_indices):
    """Pure-numpy static plan: edge chunking schedule, SelT matrices, gather index
    arrays, AllToAll row-exchange lists.  Returns dict of per-core arrays.

    All shapes/counts are identical across cores (max-padded) because the device
    program is SPMD: one instruction stream, per-core differences live in data.
    """
    c = _derived(cfg)
    L, NBR, B, NCORES, BLK = c["L"], c["NBR"], c["B"], c["NCORES"], c["BLK"]
    BC, NBLK = c["BC"], c["NBLK"]

    keep = edge_dst < B
    src = edge_src[keep].astype(np.int64)
    dst = edge_dst[keep].astype(np.int64)
    w = edge_weight[keep].astype(np.float32)

    owner = dst // BC
    dst_local = dst - owner * BC
    blk = dst_local // BLK
    dcol = dst_local % BLK
    is_h = src < B

    # ---- per (core, blk) edge index lists
    h_edges = [[None] * NBLK for _ in range(NCORES)]
    fo_edges = [[None] * NBLK for _ in range(NCORES)]
    for j in range(NCORES):
        mj = owner == j
        for b in range(NBLK):
            m = mj & (blk == b)
            h_edges[j][b] = np.flatnonzero(m & is_h)
            fo_edges[j][b] = np.flatnonzero(m & ~is_h)

    # ---- chunk schedule (shared across cores: max over cores per block)
    nh_ch = [max(math.ceil(len(h_edges[j][b]) / 128) for j in range(NCORES))
             for b in range(NBLK)]
    nf_ch = [max(math.ceil(len(fo_edges[j][b]) / 128) for j in range(NCORES))
             for b in range(NBLK)]
    # global chunk table: per block, h-chunks then fo-chunks
    sched = []  # (block, kind, within-kind sequence index)
    h_seq = f_seq = 0
    for b in range(NBLK):
        for _ in range(nh_ch[b]):
            sched.append((b, "h", h_seq)); h_seq += 1
        for _ in range(nf_ch[b]):
            sched.append((b, "fo", f_seq)); f_seq += 1
    NCH = len(sched)
    NHC, NFC = max(h_seq, 1), max(f_seq, 1)

    # ---- AllToAll compact table: rows_from[i][j] = sorted h rows owned by i, needed by j
    need = []
    for j in range(NCORES):
        idx = np.concatenate([h_edges[j][b] for b in range(NBLK)]) \
            if NBLK else np.zeros(0, np.int64)
        need.append(np.unique(src[idx.astype(np.int64)]) if len(idx) else
                    np.zeros(0, np.int64))
    rows_from = [[None] * NCORES for _ in range(NCORES)]
    for j in range(NCORES):
        ow = need[j] // BC
        for i in range(NCORES):
            rows_from[i][j] = need[j][ow == i]
    S = max(max(len(rows_from[i][j]) for j in range(NCORES)) for i in range(NCORES))
    S = max(16, ((S + 15) // 16) * 16)     # 8*S % 128 == 0 so TAB fills whole chunks
    TAB = NCORES * S
    NSEND_CH = TAB // 128

    # position-of-row lookup per receiver
    pos_of_row = np.zeros((NCORES, B), np.int64)
    for j in range(NCORES):
        for i in range(NCORES):
            r = rows_from[i][j]
            pos_of_row[j, r] = i * S + np.arange(len(r))

    plan = dict(cfg=c, NCH=NCH, NHC=NHC, NFC=NFC, S=S, TAB=TAB,
                NSEND_CH=NSEND_CH, sched=sched, nh_ch=nh_ch, nf_ch=nf_ch)

    # ---- per-core arrays (device layouts: partition-major / wrapped int16)
    selT = np.zeros((NCORES, 128, NCH, BLK), np.float32)   # [p, chunk, dstcol]
    h_flat = np.zeros((NCORES, NHC * 128), np.int64)       # edge slot -> table row
    M = cfg["M"]
    fo_flat = np.zeros((NCORES, L, NFC * NBR * 128), np.int64)
    send_idx = np.zeros((NCORES, 128, NSEND_CH), np.int32)

    for j in range(NCORES):
        q = 0
        for b in range(NBLK):
            for kind, nch, elist in (("h", nh_ch[b], h_edges[j][b]),
                                     ("fo", nf_ch[b], fo_edges[j][b])):
                if nch == 0:
                    continue
                seq0 = sched[q][2]
                t = np.arange(len(elist))
                cl = t // 128
                p = t % 128
                selT[j, p, q + cl, dcol[elist]] = w[elist]
                if kind == "h":
                    h_flat[j, (seq0 + cl) * 128 + p] = pos_of_row[j, src[elist]]
                else:
                    fon = src[elist] - B
                    fi = first_order_idx[fon]
                    for l in range(L):
                        for br in range(NBR):
                            fo_flat[j, l, (seq0 + cl) * NBR * 128
                                    + br * 128 + p] = br * M + c_indices[l, br, fi]
                q += nch
        assert q == NCH
        sl = np.zeros(TAB, np.int64)
        for jj in range(NCORES):
            r = rows_from[j][jj] - j * BC
            sl[jj * S: jj * S + len(r)] = r
        send_idx[j] = sl.reshape(NSEND_CH, 128).T

    def wrap16(flat):
        # [n] -> [128, n//16] int16: partition 16g+r, col k = flat[k*16+r]
        n = flat.shape[-1]
        a = flat.reshape(*flat.shape[:-1], n // 16, 16)
        a = np.moveaxis(a, -1, -2)          # [..., 16, n//16]
        return np.ascontiguousarray(
            np.concatenate([a] * 8, axis=-2)).astype(np.int16)

    plan["selT"] = np.ascontiguousarray(
        selT.reshape(NCORES, 128, NCH * BLK)).astype(BF16)
    plan["h_idx16"] = wrap16(h_flat)                       # [NC,128,NHC*8]
    plan["fo_idx16"] = wrap16(fo_flat)                     # [NC,L,128,NFC*NBR*8]
    plan["send_idx16"] = wrap16(
        send_idx.T.reshape(NCORES, -1) if False else
        np.stack([send_idx[j].T.reshape(-1) for j in range(NCORES)]))
    plan["rows_from"] = rows_from
    return plan


def fold_weights(cfg, codebooks, Wc, bc, Wt, bt, Ws, bs, Wf, bf):
    L, C = cfg["L"], cfg["C"]
    Wct = np.stack([Wc[l] @ Wt[l] for l in range(L)])             # [L,C,C]
    bias = np.stack([bc[l] @ Wt[l] + bt[l] + bs[l] for l in range(L)])
    # dense rhs layout [128, L*4*C]: per layer: Wct h0, Wct h1, Ws h0, Ws h1
    wd = np.zeros((128, L, 4, C), np.float32)
    for l in range(L):
        wd[:, l, 0] = Wct[l][:128]
        wd[:, l, 1] = Wct[l][128:]
        wd[:, l, 2] = Ws[l][:128]
        wd[:, l, 3] = Ws[l][128:]
    wf = np.stack([Wf[:128], Wf[128:]], axis=1)                    # [128,2,C]
    biases = np.concatenate([bias, bf[None, :]], 0)                # [L+1, C]
    cb_feat = codebooks[:, :, :, :cfg["D"]]                        # [L,NBR,M,D]
    cb_all = cb_feat.reshape(L, cfg["NBR"] * cfg["M"], cfg["D"])   # [L,4M,D]
    return (np.ascontiguousarray(wd.reshape(128, L * 4 * C)).astype(BF16),
            np.ascontiguousarray(wf.reshape(128, 2 * C)).astype(BF16),
            np.ascontiguousarray(biases.reshape(1, (L + 1) * C)).astype(BF16),
            np.ascontiguousarray(cb_all).astype(np.float32))


# ---------------------------------------------------------------- device kernel
def build_kernel(plan):
    c = plan["cfg"]
    L, NBR, Csz, Dsz, Msz = c["L"], c["NBR"], c["C"], c["D"], c["M"]
    NCORES, BLK, NBLK, BCP = c["NCORES"], c["BLK"], c["NBLK"], c["BCP"]
    NCH, NHC, NFC, TAB, NSEND_CH = (plan["NCH"], plan["NHC"], plan["NFC"],
                                    plan["TAB"], plan["NSEND_CH"])
    sched, nh_ch, nf_ch = plan["sched"], plan["nh_ch"], plan["nf_ch"]
    WINB = c["WIN_BLOCKS"]
    FP32, BF, I32 = mybir.dt.float32, mybir.dt.bfloat16, mybir.dt.int32

    nc = bacc.Bacc("TRN2", target_bir_lowering=False, debug=False,
                   num_devices=NCORES)

    # ---- external inputs (per-core)
    I16 = mybir.dt.int16
    selT_d = nc.dram_tensor("selT", [128, NCH * BLK], BF, kind="ExternalInput")
    h_idx_d = nc.dram_tensor("h_idx16", [128, NHC * 8], I16, kind="ExternalInput")
    fo_idx_d = nc.dram_tensor("fo_idx16", [L, 128, NFC * NBR * 8], I16,
                              kind="ExternalInput")
    send_idx_d = nc.dram_tensor("send_idx16", [128, TAB // 16], I16,
                                kind="ExternalInput")
    cb_d = [nc.dram_tensor(f"cb_{l}", [NBR * Msz, Dsz], FP32,
                           kind="ExternalInput") for l in range(L)]
    wd_d = nc.dram_tensor("wd", [128, L * 4 * Csz], BF, kind="ExternalInput")
    wf_d = nc.dram_tensor("wf", [128, 2 * Csz], BF, kind="ExternalInput")
    bias_d = nc.dram_tensor("biases", [1, (L + 1) * Csz], BF, kind="ExternalInput")
    x_compact_d = nc.dram_tensor("x_compact", [TAB, Csz], BF, kind="ExternalInput")
    h_local0_d = nc.dram_tensor("h_local0", [BCP, Csz], BF, kind="ExternalInput")
    y_d = nc.dram_tensor("y", [BCP, Csz], FP32, kind="ExternalOutput")

    # ---- window partition of the chunk schedule (by blocks); within a window the
    # msgs buffer holds all h-chunks first, then all fo-chunks -> one batched
    # indirect gather per kind (per branch for fo) per window.
    NWIN = math.ceil(NBLK / WINB)
    win_chunks = [[] for _ in range(NWIN)]     # ordered (q, b, kind, seq)
    for q, (b, kind, seq) in enumerate(sched):
        win_chunks[b // WINB].append((q, b, kind, seq))
    win_layout = []   # per window: (hw list, fw list)
    for wI in range(NWIN):
        hw = [x for x in win_chunks[wI] if x[2] == "h"]
        fw = [x for x in win_chunks[wI] if x[2] == "fo"]
        win_layout.append((hw, fw))
    max_nh = max(len(hw) for hw, fw in win_layout)
    max_nfo = max(len(fw) for hw, fw in win_layout)

    with tile.TileContext(nc) as tc:
        with (
            tc.tile_pool(name="const", bufs=1) as constp,
            tc.tile_pool(name="win", bufs=2) as winp,
            tc.tile_pool(name="idx", bufs=3) as idxp,
            tc.tile_pool(name="segps", bufs=2, space="PSUM") as segp,
            tc.tile_pool(name="outps", bufs=3, space="PSUM") as outp,
            tc.tile_pool(name="seg_sb", bufs=3) as segsb,
            tc.tile_pool(name="self32", bufs=6) as selfp,
            tc.tile_pool(name="ht", bufs=4) as htp,
            tc.tile_pool(name="out_sb", bufs=3) as outsb,
            tc.tile_pool(name="stage", bufs=1) as stagep,
            tc.tile_pool(name="dram", bufs=1, space="DRAM") as dramp,
        ):
            # ---- resident constants
            selT_sb = constp.tile([128, NCH * BLK], BF, name="selT_sb")
            nc.sync.dma_start(out=selT_sb[:], in_=selT_d[:])
            wd_sb = constp.tile([128, L * 4 * Csz], BF, name="wd_sb")
            nc.sync.dma_start(out=wd_sb[:], in_=wd_d[:])
            wf_sb = constp.tile([128, 2 * Csz], BF, name="wf_sb")
            nc.sync.dma_start(out=wf_sb[:], in_=wf_d[:])
            bias_sb = constp.tile([1, (L + 1) * Csz], BF, name="bias_sb")
            nc.sync.dma_start(out=bias_sb[:], in_=bias_d[:])
            ones_sb = constp.tile([1, 128], BF, name="ones_sb")
            nc.vector.memset(ones_sb[:], 1.0)

            # ---- DRAM internals
            h_locals = [h_local0_d[:]]
            for l in range(1, L + 1):
                t = dramp.tile([BCP, Csz], BF, name=f"h_local{l}")
                h_locals.append(t)
            xh_tabs = [x_compact_d[:]]
            for l in range(1, L):
                t = dramp.tile([TAB, Csz], BF, name=f"xh_tab{l}")
                xh_tabs.append(t)
            a2a_in = dramp.tile([TAB, Csz], BF, name="a2a_in")

            def wslice(l, k):          # dense rhs [128, C]
                return wd_sb[:, (l * 4 + k) * Csz: (l * 4 + k + 1) * Csz]

            def bslice(l):
                return bias_sb[:, l * Csz: (l + 1) * Csz]

            for l in range(L):
                msgs_of_chunk = {}
                for wI in range(NWIN):
                    hw, fw = win_layout[wI]
                    msgs_h = winp.tile([128, max(max_nh, 1) * Csz], BF,
                                       name="msgs_h", tag="msgs_h")
                    msgs_fo = winp.tile([128, max(max_nfo, 1) * NBR * Dsz], FP32,
                                        name="msgs_fo", tag="msgs_fo")
                    nfo = len(fw)
                    for i, x in enumerate(hw):
                        msgs_of_chunk[x[0]] = ("h", msgs_h, i, 0)
                    for i, x in enumerate(fw):
                        msgs_of_chunk[x[0]] = ("fo", msgs_fo, i, nfo)
                    if hw:
                        s0, s1 = hw[0][3], hw[-1][3] + 1
                        nh = s1 - s0
                        hidx = idxp.tile([128, nh * 8], I16, name="hidx",
                                         tag="hidx")
                        nc.sync.dma_start(out=hidx[:],
                                          in_=h_idx_d[:, s0 * 8:s1 * 8])
                        nc.gpsimd.dma_gather(
                            msgs_h[:, 0:nh * Csz]
                                .rearrange("p (k c) -> p k c", c=Csz),
                            xh_tabs[l][:, :],
                            hidx[:],
                            nh * 128, nh * 128, Csz,
                            single_packet=False,
                        )
                    if fw:
                        s0, s1 = fw[0][3], fw[-1][3] + 1
                        assert nfo == s1 - s0
                        fidx = idxp.tile([128, nfo * NBR * 8], I16, name="fidx",
                                         tag="fidx")
                        nc.sync.dma_start(
                            out=fidx[:],
                            in_=fo_idx_d[l, :, s0 * NBR * 8:s1 * NBR * 8])
                        nc.gpsimd.dma_gather(
                            msgs_fo[:, 0:nfo * NBR * Dsz]
                                .rearrange("p (k c) -> p k c", c=Dsz),
                            cb_d[l][:, :],
                            fidx[:],
                            nfo * NBR * 128, nfo * NBR * 128, Dsz,
                            single_packet=False,
                        )

                # ---- per block: scatter + dense
                q = 0
                for b in range(NBLK):
                    nch_b = nh_ch[b] + nf_ch[b]
                    segT0 = segp.tile([128, BLK], FP32, name="segT0", tag="segT0")
                    segT1 = segp.tile([128, BLK], FP32, name="segT1", tag="segT1")
                    # fo chunks first: they are independent of the inter-layer
                    # AllToAll, so their PE work overlaps the collective; only
                    # the trailing h-chunk matmuls wait on the exchanged table.
                    qgs = [q + k for k in range(nch_b)]
                    qgs = ([g for g in qgs if msgs_of_chunk[g][0] == "fo"]
                           + [g for g in qgs if msgs_of_chunk[g][0] == "h"])
                    for k in range(nch_b):
                        qg = qgs[k]
                        kind, msgs, ci, nfo_w = msgs_of_chunk[qg]
                        if kind == "h":
                            rhs = selT_sb[:, qg * BLK:(qg + 1) * BLK]
                            for half, seg in ((0, segT0), (1, segT1)):
                                nc.tensor.matmul(
                                    out=seg[:],
                                    lhsT=msgs[:, ci * Csz + half * 128:
                                              ci * Csz + half * 128 + 128],
                                    rhs=rhs,
                                    start=(k == 0), stop=(k == nch_b - 1),
                                )
                        else:
                            sel32 = selfp.tile([128, BLK], FP32, name="sel32",
                                               tag="sel32")
                            if qg % 2 == 0:
                                nc.vector.tensor_copy(
                                    out=sel32[:],
                                    in_=selT_sb[:, qg * BLK:(qg + 1) * BLK])
                            else:
                                nc.scalar.activation(
                                    sel32[:],
                                    selT_sb[:, qg * BLK:(qg + 1) * BLK],
                                    mybir.ActivationFunctionType.Copy)
                            base = ci * NBR * Dsz
                            for half, seg in ((0, segT0), (1, segT1)):
                                nc.tensor.matmul(
                                    out=seg[:],
                                    lhsT=msgs[:, base + half * 128:
                                              base + half * 128 + 128],
                                    rhs=sel32[:],
                                    start=(k == 0), stop=(k == nch_b - 1),
                                )
                    q += nch_b
                    segT_sb = segsb.tile([128, 2 * BLK], BF, name="segT_sb",
                                         tag="segT_sb")
                    nc.vector.tensor_copy(out=segT_sb[:, 0:BLK], in_=segT0[:])
                    nc.scalar.activation(segT_sb[:, BLK:2 * BLK], segT1[:],
                                         mybir.ActivationFunctionType.Copy)
                    hT = htp.tile([128, 2 * BLK], BF, name="hT", tag="hT")
                    for half in range(2):
                        nc.sync.dma_start(
                            out=hT[:, half * BLK:(half + 1) * BLK],
                            in_=h_locals[l][b * BLK:(b + 1) * BLK,
                                            half * 128:(half + 1) * 128],
                            transpose=True)
                    out_ps = outp.tile([128, Csz], FP32, name="out_ps",
                                       tag="out_ps")
                    nc.tensor.matmul(out=out_ps[:], lhsT=segT_sb[:, 0:BLK],
                                     rhs=wslice(l, 0), start=True, stop=False)
                    nc.tensor.matmul(out=out_ps[:], lhsT=segT_sb[:, BLK:2 * BLK],
                                     rhs=wslice(l, 1), start=False, stop=False)
                    nc.tensor.matmul(out=out_ps[:], lhsT=hT[:, 0:BLK],
                                     rhs=wslice(l, 2), start=False, stop=False)
                    nc.tensor.matmul(out=out_ps[:], lhsT=hT[:, BLK:2 * BLK],
                                     rhs=wslice(l, 3), start=False, stop=False)
                    nc.tensor.matmul(out=out_ps[:], lhsT=ones_sb[:, :],
                                     rhs=bslice(l), start=False, stop=True)
                    out_sb = outsb.tile([128, Csz], BF, name="out_sb",
                                        tag="out_sb")
                    fn = (mybir.ActivationFunctionType.Relu if l < L - 1
                          else mybir.ActivationFunctionType.Copy)
                    nc.scalar.activation(out_sb[:], out_ps[:], fn)
                    nc.sync.dma_start(out=h_locals[l + 1][b * BLK:(b + 1) * BLK, :],
                                      in_=out_sb[:])

                # ---- exchange for next layer
                if l < L - 1:
                    sidx = idxp.tile([128, TAB // 16], I16, name="sidx",
                                     tag="sidx")
                    nc.sync.dma_start(out=sidx[:], in_=send_idx_d[:])
                    stg = stagep.tile([128, NSEND_CH * Csz], BF, name="stg")
                    nc.gpsimd.dma_gather(
                        stg[:].rearrange("p (k c) -> p k c", c=Csz),
                        h_locals[l + 1][:, :],
                        sidx[:],
                        TAB, TAB, Csz,
                        single_packet=False,
                    )
                    nc.sync.dma_start(
                        out=a2a_in[:].rearrange("(k p) c -> p k c", p=128),
                        in_=stg[:].rearrange("p (k c) -> p k c", c=Csz))
                    nc.gpsimd.collective_compute(
                        "AllToAll", mybir.AluOpType.bypass,
                        replica_groups=[list(range(NCORES))],
                        ins=[a2a_in[:]],
                        outs=[xh_tabs[l + 1][:]],
                    )

            # ---- final layer: y = h3 @ Wf + bf
            for b in range(NBLK):
                hT = htp.tile([128, 2 * BLK], BF, name="hTf", tag="hT")
                for half in range(2):
                    nc.sync.dma_start(
                        out=hT[:, half * BLK:(half + 1) * BLK],
                        in_=h_locals[L][b * BLK:(b + 1) * BLK,
                                        half * 128:(half + 1) * 128],
                        transpose=True)
                out_ps = outp.tile([128, Csz], FP32, name="out_psf", tag="out_ps")
                nc.tensor.matmul(out=out_ps[:], lhsT=hT[:, 0:BLK],
                                 rhs=wf_sb[:, 0:Csz], start=True, stop=False)
                nc.tensor.matmul(out=out_ps[:], lhsT=hT[:, BLK:2 * BLK],
                                 rhs=wf_sb[:, Csz:2 * Csz], start=False, stop=False)
                nc.tensor.matmul(out=out_ps[:], lhsT=ones_sb[:, :],
                                 rhs=bslice(L), start=False, stop=True)
                y_sb = outsb.tile([128, Csz], FP32, name="y_sb", tag="y_sb")
                nc.scalar.activation(y_sb[:], out_ps[:],
                                     mybir.ActivationFunctionType.Copy)
                nc.sync.dma_start(out=y_d[b * BLK:(b + 1) * BLK, :], in_=y_sb[:])

    nc.compile()
    return nc


# ---------------------------------------------------------------- entry point
def prep_inputs(cfg, inputs):
    c = _derived(cfg)
    plan = make_plan(cfg, inputs["first_order_idx"], inputs["edge_src"],
                     inputs["edge_dst"], inputs["edge_weight"],
                     inputs["c_indices"])
    wd, wf, biases, cb = fold_weights(
        cfg, np.asarray(inputs["codebooks"]), np.asarray(inputs["Wc"]),
        np.asarray(inputs["bc"]), np.asarray(inputs["Wt"]),
        np.asarray(inputs["bt"]), np.asarray(inputs["Ws"]),
        np.asarray(inputs["bs"]), np.asarray(inputs["Wf"]),
        np.asarray(inputs["bf"]))
    x = np.asarray(inputs["x"], dtype=np.float32)
    NCORES, BC, BCP, S = c["NCORES"], c["BC"], c["BCP"], plan["S"]
    in_maps = []
    for j in range(NCORES):
        tabrows = np.zeros(plan["TAB"], np.int64)
        for i in range(NCORES):
            r = plan["rows_from"][i][j]
            tabrows[i * S: i * S + len(r)] = r
        x_comp = np.ascontiguousarray(x[tabrows]).astype(BF16)
        h0 = np.zeros((BCP, cfg["C"]), BF16)
        h0[:BC] = x[j * BC:(j + 1) * BC].astype(BF16)
        in_maps.append({
            "selT": plan["selT"][j],
            "h_idx16": plan["h_idx16"][j],
            "fo_idx16": plan["fo_idx16"][j],
            "send_idx16": plan["send_idx16"][j],
            **{f"cb_{l}": np.ascontiguousarray(cb[l]) for l in range(cfg["L"])},
            "wd": wd, "wf": wf, "biases": biases,
            "x_compact": x_comp, "h_local0": h0,
        })
    return plan, in_maps


_NC_CACHE = {}


def get_nc(plan):
    key = (plan["NCH"], plan["NHC"], plan["NFC"], plan["TAB"],
           tuple(plan["nh_ch"]), tuple(plan["nf_ch"]))
    if key not in _NC_CACHE:
        _NC_CACHE[key] = build_kernel(plan)
    return _NC_CACHE[key]


def kernel(**inputs):
    cfg = CFG
    c = _derived(cfg)
    plan, in_maps = prep_inputs(cfg, inputs)
    nc = get_nc(plan)
    res = run_bass_kernel_spmd(nc, in_maps, list(range(cfg["NCORES"])))
    B, BC, C = cfg["B"], c["BC"], cfg["C"]
    y = np.zeros((B, C), np.float32)
    for j in range(cfg["NCORES"]):
        y[j * BC:(j + 1) * BC] = res.results[j]["y"][:BC]
    return y
